# revision 38
# baseline (speedup 1.0000x reference)
"""Trainium2 Bass kernel for top-1 MoE expert MLP (nn_Experts problem).

Strategy (expert-parallel, one expert per NeuronCore):
  - Routing is one-hot top-1: each token is processed by exactly one expert,
    so each core computes the MLP only for the tokens routed to its expert.
  - Host-side shard step: compute token->expert assignment from
    dispatch_tensor, gather each expert's tokens (transposed to [D, CAP]),
    cast everything to bf16 (rel err ~4e-3 << the 2e-2 gate), and pack
    weights into per-tile blocks so every DMA row is contiguous and large.
  - Device phase A: h^T[F, CAP] = gelu(w1^T @ x^T + b1)    (moving = tokens)
  - Device phase B: y^T[D, CAP] = (w2^T @ h^T) * gate      (moving = tokens)
  - Host-side unshard: scatter y^T columns back to token order, add b2.

DMA architecture (from perfetto-trace analysis; 80.7us baseline -> ~76us):
  - Fabric is ~270-330GB/s per core while all 8 cores fetch
    simultaneously.  The critical path is head (x k0 + w1 m0 + b1) + x
    (1.45MB): m1=0 cannot finish before ~t0+4.6us, so everything else is
    kept off the fabric until then.
  - The two HWDGE rings (sync, scalar) expand descriptors with DIRECT2D
    instructions executed inline on their sequencer streams; a bulk
    transfer queued behind a critical one stalls on the small HW ring FIFO
    and delays the critical transfer's completion-event forwarding by
    microseconds.  The SWDGE (gpsimd) ring has a deep software FIFO
    (expansions never stall, events fire at data arrival) but ~2-3.5us
    release-to-first-data latency.  So: HWDGE carries ONLY head+x (sync:
    head, xc; scalar: xa, xb) and the y writes; ALL weights ride SWDGE.
  - The scheduler hoists dependency-free dma_starts to the front of their
    queue, so late emission cannot sequence transfers.  Real WAR gates
    (memset the dest corner; a tiny gpsimd mul reads it, chained off an x
    corner) hold the bulk weight stream back: w1_0 is ungated (m1=1 needs
    it immediately after m1=0 and SWDGE latency is ~3us), w1_1 releases on
    xa, w1_2a.. + w2 + gbg on xb.  Block sizes (1,2,2,2,4,4) m-tiles give
    2-8KB rows: early blocks arrive just-in-time, later fat-row blocks
    sustain 300-400GB/s.
  - m1=0 and m1=1 are interleaved, consuming tiles in arrival order
    (PSUM accumulation order is free): m0 k0-k2 (head/xa), m1 k0-k2
    (w1_0), m0 k5-k7 (xc), m0 k3-k4 (xb), m1 k3-k7 — so the x-wait
    window is filled with real work instead of fillers.
  - y writes ride the (idle during phase B) sync ring; the last d-tile is
    split 384/128 so minimal work remains after the final matmul.

PE warm-up: the HAM clock gate keeps the PE at 1.2 GHz until it has been
busy for a ~3.4us activity window, and any idle gap >3.4us re-throttles it
(idle time freezes ramp progress).  Dummy matmuls on a (gpsimd-memset)
scratch tile start right at "main" and run until the head transfer lands,
so real work begins at 2.4 GHz; filler dummies cover the early data waits.
A dummy activation right after the dma_starts forces both gelu ACT tables
to load at ~6-9us (concurrent, scalar engine idle) instead of mid-phase-A.
"""

import numpy as np

B, N, D, E, F = 8, 512, 1024, 8, 2048
T = B * N
P = 128
CAP = 512            # per-expert token capacity.  Seed-0 max count is 549;
                     # the ~90 overflow tokens (capacity factor 1.0) take
                     # the exact host fallback path.  CAP=512 makes every
                     # matmul a single full-PSUM-bank 512-wide pass.
KT1 = D // P         # 8  k-tiles for matmul1 (contract over D)
MT1 = F // P         # 16 m-tiles for matmul1 / k-tiles for matmul2
DT1 = D // P         # 8  output d-tiles for matmul2
C0 = CAP
CHUNKS = ((0, CAP),)
W1_BLOCKS = (1, 2, 2, 2, 4, 4)       # m1 = 1..15 DMA block sizes
W1_OFF = (1, 2, 4, 6, 8, 12)
W1_XB_GATE = 2       # w1 blocks >= this index release on xb; earlier
                     # blocks release on xa (small early fabric steal,
                     # arrive just-in-time for m1..m3)
X_SPLIT = ((0, 2), (2, 4), (4, 7))   # xa (scalar), xb (scalar), xc (sync)
N_WARM = 18          # upfront dummy matmuls: keep PE busy from ~6.2us
                     # ("main") until head lands (~9.9-11us; sized for
                     # slow-DMA cores) so the HAM clock is at 2.4GHz when
                     # real work starts.
# Gap-filler dummies at phase-A stall points for m1 >= 2 (none needed in
# steady state; the m1=0/1 interleave below has its own hardcoded
# fillers).  {(m1, k): fillers after k-step k, (m1, None): after group}.
FILLERS = {}
W_FILL = 256         # moving width of warm-up/filler dummies

_NC_CACHE = {}


def _build_bass():
    import concourse.bacc as bacc
    import concourse.tile as tile
    from concourse import mybir

    f32 = mybir.dt.float32
    bf16 = mybir.dt.bfloat16

    nc = bacc.Bacc(None, target_bir_lowering=False)
    # head packs xT k-block 0, the whole w1 m1=0 block (all 8 k-tiles), and
    # the b1 bias columns: one early 3.1KB-row transfer covers everything
    # m1=0 needs to start.
    HW = CAP + D + MT1
    head = nc.declare_dram_parameter("head", [P, HW], bf16, isOutput=False)
    xr = nc.declare_dram_parameter("xr", [P, KT1 - 1, CAP], bf16, isOutput=False)
    w1r = nc.declare_dram_parameter("w1r", [P, MT1 - 1, D], bf16, isOutput=False)
    w2r = nc.declare_dram_parameter("w2r", [P, MT1, D], bf16, isOutput=False)
    gbg = nc.declare_dram_parameter("gbg", [P, CAP], f32, isOutput=False)
    y = nc.declare_dram_parameter("y", [D, CAP], bf16, isOutput=True)

    with tile.TileContext(nc) as tc:
        with (
            tc.tile_pool(name="wp", bufs=1) as wp,
            tc.tile_pool(name="hdp", bufs=1) as hdp,
            tc.tile_pool(name="xp", bufs=1) as xp,
            tc.tile_pool(name="w1p", bufs=1) as w1p,
            tc.tile_pool(name="w2p", bufs=1) as w2p,
            tc.tile_pool(name="gp", bufs=1) as gp,
            tc.tile_pool(name="hp", bufs=MT1) as hp,
            tc.tile_pool(name="stp", bufs=3) as stp,
            tc.tile_pool(name="psA", bufs=4, space="PSUM") as psA,
            tc.tile_pool(name="psB", bufs=4, space="PSUM") as psB,
        ):
            # --- PE warm-up: memset + dummy matmuls emitted BEFORE any
            # dma_start, so the memset-done event reaches the PE before the
            # gpsimd sequencer starts expanding SWDGE descriptors. ---
            warm = wp.tile([P, W_FILL], bf16, tag="warm")
            nc.gpsimd.memset(warm[:], 0.0)
            wps = psB.tile([P, C0], f32, tag="psB0", name="warm_ps")
            # tiny psum (borrows a psA slot; phase A's rotation reaches it
            # only at m1=3, long after the dummy act below has read it)
            # feeding the early dummy activation without depending on the
            # warm matmul chain
            tiny = psA.tile([2, 256], f32, tag="psA0", name="tiny_ps")
            nc.tensor.matmul(tiny[:, :], warm[:, 0:2], warm[:],
                             start=True, stop=True, skip_group_check=True)
            for i in range(N_WARM):
                nc.tensor.matmul(wps[:, :W_FILL], warm[:, 0:P], warm[:],
                                 start=True, stop=True, skip_group_check=True)

            # --- DMA plan (see module docstring) ---
            head_t = hdp.tile([P, HW], bf16, tag="head")
            nc.sync.dma_start(out=head_t[:], in_=head[:, :])

            x_t = []
            for j, (a, b) in enumerate(X_SPLIT):
                x_t.append(xp.tile([P, b - a, CAP], bf16, tag=f"x{j}",
                                   name=f"x{j}"))
            nc.scalar.dma_start(out=x_t[0][:], in_=xr[:, 0:2, :])
            nc.scalar.dma_start(out=x_t[1][:], in_=xr[:, 2:4, :])
            nc.sync.dma_start(out=x_t[2][:], in_=xr[:, 4:7, :])

            w1_t = []
            for j, nm in enumerate(W1_BLOCKS):
                w1_t.append(w1p.tile([P, nm, D], bf16, tag=f"w1_{j}",
                                     name=f"w1_{j}"))
            w2_t = [w2p.tile([P, 8, D], bf16, tag=f"w2_{j}",
                             name=f"w2_{j}") for j in range(2)]

            def load_w1(j):
                r0 = W1_OFF[j] - 1
                nc.gpsimd.dma_start(out=w1_t[j][:],
                                    in_=w1r[:, r0:r0 + W1_BLOCKS[j], :])

            # Gate the bulk w1/w2/gbg stream on x's arrival so the fabric
            # (~330GB/s under 8-core load) stays mostly head+x's until
            # m1=0's data is in.  A gate must be a REAL dependency: each
            # transfer's destination corner is memset, then a tiny gpsimd
            # mul reads that corner (WAR: the DMA overwrite must wait for
            # the read) chained off the x corner, because the scheduler
            # hoists dependency-free dma_starts to the front of their
            # queue.  SWDGE release-to-first-data latency is ~2-3.5us, so
            # w1_0 (needed by m1=1 right after m1=0) flows UNGATED from the
            # start — its 0.26MB early fabric steal is cheaper than a late
            # m1 entry.  w1_1 releases on xa, everything else on xb.
            gb_sb = gp.tile([P, CAP], f32, tag="gbg")
            load_w1(0)
            gate_seq = [w1_t[j][0:2, 0, 0:2]
                        for j in range(1, len(W1_BLOCKS))]
            gate_seq += [w2_t[0][0:2, 0, 0:2], w2_t[1][0:2, 0, 0:2],
                         gb_sb[0:2, 0:2]]
            for c in gate_seq:
                nc.gpsimd.memset(c, 0.0)
            gate_s = wp.tile([2, 2 * (len(gate_seq) + 1)], f32, tag="gate_s")

            def gate(i, trigger):
                nc.gpsimd.tensor_mul(gate_s[:, 2 * i:2 * i + 2], trigger,
                                     gate_seq[i])

            for j in range(1, len(W1_BLOCKS)):
                i = j - 1
                if j == 1:
                    trig = x_t[0][0:2, 0, 0:2]            # <- xa
                elif j == W1_XB_GATE:
                    trig = x_t[1][0:2, 0, 0:2]            # <- xb
                else:
                    trig = gate_s[:, 2 * i - 2:2 * i]     # chain
                gate(i, trig)
                load_w1(j)
            nw = len(W1_BLOCKS) - 1
            for j in range(2):
                gate(nw + j, gate_s[:, 2 * (nw + j) - 2:2 * (nw + j)])
                nc.gpsimd.dma_start(out=w2_t[j][:],
                                    in_=w2r[:, j * 8:(j + 1) * 8, :])
            gate(nw + 2, gate_s[:, 2 * (nw + 2) - 2:2 * (nw + 2)])
            nc.gpsimd.dma_start(out=gb_sb[:], in_=gbg[:, :])

            # Dummy activation: forces both gelu ACT tables to load on the
            # scalar ENGINE now (~2.6us, concurrent with DMA descriptor
            # expansion on the scalar SEQUENCER), instead of the second
            # table load landing mid-phase-A where it delays event
            # forwarding on the scalar ring.
            gelu = mybir.ActivationFunctionType.Gelu
            act_s = wp.tile([2, 2], f32, tag="act_s")
            nc.scalar.activation(act_s[:, :], tiny[0:2, 0:2], gelu)

            def x_mv(k, a, b):
                if k == 0:
                    return head_t[:, a:b]
                for j, (ka, kb) in enumerate(X_SPLIT):
                    if ka <= k - 1 < kb:
                        return x_t[j][:, k - 1 - ka, a:b]

            def w1_lhs(m1, k):
                if m1 == 0:
                    return head_t[:, CAP + k * P:CAP + (k + 1) * P]
                j = next(i for i in range(len(W1_BLOCKS))
                         if W1_OFF[i] <= m1 < W1_OFF[i] + W1_BLOCKS[i])
                return w1_t[j][:, m1 - W1_OFF[j], k * P:(k + 1) * P]

            # --- Phase A: h^T[F, CAP] = gelu(w1^T @ x^T + b1) ---
            h_sb = []

            def filler(n):
                for _ in range(n):
                    nc.tensor.matmul(wps[:, :W_FILL], warm[:, 0:P], warm[:],
                                     start=True, stop=True,
                                     skip_group_check=True)

            def a_mm(ps, m1, k, st, sp):
                nc.tensor.matmul(ps[:], w1_lhs(m1, k), x_mv(k, 0, CAP),
                                 start=st, stop=sp)

            def a_gelu(ps, m1):
                h = hp.tile([P, CAP], bf16, tag="h", name=f"h_{m1}")
                bias = head_t[:, CAP + D + m1:CAP + D + m1 + 1]
                nc.scalar.activation(h[:, :], ps[:], gelu, bias=bias)
                h_sb.append(h)

            # m1=0 and m1=1 interleaved: while m1=0 waits for the late x
            # pieces (xc ~12.5us, xb ~13us), m1=1's k0..k2 (head/xa data +
            # the ungated w1_0 block, all in SBUF by ~11.3us) run as real
            # work in the wait window instead of fillers.
            ps0 = psA.tile([P, CAP], f32, tag="psA0", name="psA0_0")
            ps1 = psA.tile([P, CAP], f32, tag="psA0", name="psA0_1")
            for k in (0, 1, 2):
                a_mm(ps0, 0, k, k == 0, False)
            filler(4)
            for k in (0, 1, 2):
                a_mm(ps1, 1, k, k == 0, False)
            filler(6)
            for k in (5, 6, 7):
                a_mm(ps0, 0, k, False, False)
            filler(2)
            a_mm(ps0, 0, 3, False, False)
            a_mm(ps0, 0, 4, False, True)
            a_gelu(ps0, 0)
            for k in (3, 4, 5, 6, 7):
                a_mm(ps1, 1, k, False, k == 7)
            a_gelu(ps1, 1)
            filler(2)

            for m1 in range(2, MT1):
                pss = psA.tile([P, CAP], f32, tag="psA0",
                               name=f"psA0_{m1}")
                for i_k, k in enumerate(range(KT1)):
                    a_mm(pss, m1, k, i_k == 0, i_k == KT1 - 1)
                    filler(FILLERS.get((m1, i_k), 0))
                filler(FILLERS.get((m1, None), 0))
                a_gelu(pss, m1)

            # --- Phase B: y^T[D, CAP] = (w2^T @ h^T) * gate ---
            def b_mm(pt, d, k2, a, b):
                lhs = w2_t[k2 // 8][:, k2 % 8, d * P:(d + 1) * P]
                nc.tensor.matmul(pt[:], lhs, h_sb[k2][:, a:b],
                                 start=(k2 == 0), stop=(k2 == MT1 - 1))

            for d in range(DT1 - 1):
                pss = [psB.tile([P, b - a], f32, tag=f"psB{i}",
                                name=f"psB{i}_b{d}")
                       for i, (a, b) in enumerate(CHUNKS)]
                for k2 in range(MT1):
                    for i, (a, b) in enumerate(CHUNKS):
                        b_mm(pss[i], d, k2, a, b)
                stage = stp.tile([P, CAP], bf16, tag="stage", name=f"st_{d}")
                for i, (a, b) in enumerate(CHUNKS):
                    nc.vector.tensor_mul(stage[:, a:b], pss[i][:],
                                         gb_sb[:, a:b])
                nc.sync.dma_start(out=y[d * P:(d + 1) * P, :], in_=stage[:])
            # last tile: two sequential k2 loops, asymmetric (384/128) so
            # the work remaining after the 384-chunk's write issues is
            # minimal.  Both chunks drain on the sync ring (keeping the
            # scalar sequencer at the end barrier early).
            d = DT1 - 1
            stage = stp.tile([P, CAP], bf16, tag="stage", name=f"st_{d}")
            H = 384
            tail_eng = (nc.sync, nc.sync)
            for i, (a, b) in enumerate(((0, H), (H, CAP))):
                pt = psB.tile([P, CAP], f32, tag="psB0",
                              name=f"psB0_b{d}{i}")
                for k2 in range(MT1):
                    lhs = w2_t[k2 // 8][:, k2 % 8, d * P:(d + 1) * P]
                    nc.tensor.matmul(pt[:, a:b], lhs, h_sb[k2][:, a:b],
                                     start=(k2 == 0), stop=(k2 == MT1 - 1))
                nc.vector.tensor_mul(stage[:, a:b], pt[:, a:b],
                                     gb_sb[:, a:b])
                tail_eng[i].dma_start(out=y[d * P:(d + 1) * P, a:b],
                                      in_=stage[:, a:b])
    if not nc.is_finalized():
        nc.finalize()
    return nc


def _get_nc():
    if "nc" not in _NC_CACHE:
        _NC_CACHE["nc"] = _build_bass()
    return _NC_CACHE["nc"]


def kernel(x, dispatch_tensor, combine_tensor, w1, b1, w2, b2, **_):
    from concourse.bass_utils import run_bass_kernel_spmd
    from concourse import mybir

    bf = mybir.dt.np(mybir.dt.bfloat16)

    x = np.ascontiguousarray(np.asarray(x, dtype=np.float32)).reshape(T, D)
    dispatch = np.asarray(dispatch_tensor, dtype=np.float32).reshape(T, E)
    combine = np.asarray(combine_tensor, dtype=np.float32).reshape(T, E)
    w1 = np.asarray(w1, dtype=np.float32)
    b1 = np.asarray(b1, dtype=np.float32)
    w2 = np.asarray(w2, dtype=np.float32)
    b2 = np.asarray(b2, dtype=np.float32)

    top = dispatch.argmax(-1)
    gate = combine.sum(-1)
    full = [np.nonzero(top == e)[0] for e in range(E)]
    idxs = [idx[:CAP] for idx in full]
    spill = [idx[CAP:] for idx in full]  # ~90 tokens for the seed-0 routing

    in_maps = []
    for e in range(E):
        idx = idxs[e]
        c = len(idx)
        xT = np.zeros((D, CAP), bf)
        xT[:, :c] = x[idx].T.astype(bf)
        # w1s[m1, p, k*P+m] = w1[k*P+p, m1*P+m]: per-m1 [P, D] blocks whose
        # [:, k*P:(k+1)*P] slice is the lhsT k-tile for output tile m1.
        w1s = np.ascontiguousarray(
            w1[e].reshape(KT1, P, MT1, P).transpose(2, 1, 0, 3)
        ).astype(bf).reshape(MT1, P, D)
        # w2s[k2, p, d] = w2[k2*P+p, d]: lhsT tiles for phase B.
        w2s = np.ascontiguousarray(
            w2[e].reshape(MT1, P, D)).astype(bf).transpose(1, 0, 2)
        g = np.zeros(CAP, np.float32)
        g[:c] = gate[idx]
        gbgv = np.ascontiguousarray(
            np.broadcast_to(g[None, :], (P, CAP)))
        b1t = b1[e].reshape(MT1, P).T.astype(bf)  # [P, MT1] bias columns
        in_maps.append({
            "head": np.ascontiguousarray(
                np.concatenate([xT[:P], w1s[0], b1t], axis=1)),
            "xr": np.ascontiguousarray(
                xT[P:].reshape(KT1 - 1, P, CAP).transpose(1, 0, 2)),
            "w1r": np.ascontiguousarray(w1s[1:].transpose(1, 0, 2)),
            "w2r": np.ascontiguousarray(w2s),
            "gbg": gbgv,
        })

    global _LAST_IN_MAPS
    _LAST_IN_MAPS = in_maps
    nc = _get_nc()
    res = run_bass_kernel_spmd(nc, in_maps, list(range(E)))
    ys = [np.asarray(res.results[e]["y"], dtype=np.float32) for e in range(E)]
    if not all(np.isfinite(ye).all() for ye in ys):
        # rare transient device glitch observed once in ~20 runs: retry once
        res = run_bass_kernel_spmd(nc, in_maps, list(range(E)))
        ys = [np.asarray(res.results[e]["y"], dtype=np.float32)
              for e in range(E)]

    y_flat = np.empty((T, D), np.float32)
    for e in range(E):
        c = len(idxs[e])
        y_flat[idxs[e]] = ys[e].T[:c]
        if len(spill[e]):
            # capacity-overflow tokens: exact fp32 math on host
            import math

            erf = np.frompyfunc(math.erf, 1, 1)
            hs = x[spill[e]] @ w1[e] + b1[e]
            hs = hs * 0.5 * (1.0 + erf(hs / np.sqrt(2.0)).astype(np.float64))
            y_flat[spill[e]] = ((hs.astype(np.float32) @ w2[e])
                                * gate[spill[e]][:, None])
    return (y_flat + b2[None, :]).reshape(B, N, D)


# revision 39
# speedup vs baseline: 1.1568x; 1.1568x over previous
"""Trainium2 Bass kernel for top-1 MoE expert MLP (nn_Experts problem).

Strategy (expert-parallel, one expert per NeuronCore):
  - Routing is one-hot top-1: each token is processed by exactly one expert,
    so each core computes the MLP only for the tokens routed to its expert.
  - Host-side shard step: compute token->expert assignment from
    dispatch_tensor, gather each expert's tokens (transposed to [D, CAP]),
    cast everything to bf16 (rel err ~4e-3 << the 2e-2 gate), and pack
    weights into per-tile blocks so every DMA row is contiguous and large.
  - Device phase A: h^T[F, CAP] = gelu(w1^T @ x^T + b1)    (moving = tokens)
  - Device phase B: y^T[D, CAP] = (w2^T @ h^T) * gate      (moving = tokens)
  - Host-side unshard: scatter y^T columns back to token order, add b2.

DMA architecture (from perfetto-trace analysis; 80.7us baseline -> ~76us):
  - Fabric is ~270-330GB/s per core while all 8 cores fetch
    simultaneously.  The critical path is head (x k0 + w1 m0 + b1) + x
    (1.45MB): m1=0 cannot finish before ~t0+4.6us, so everything else is
    kept off the fabric until then.
  - The two HWDGE rings (sync, scalar) expand descriptors with DIRECT2D
    instructions executed inline on their sequencer streams; a bulk
    transfer queued behind a critical one stalls on the small HW ring FIFO
    and delays the critical transfer's completion-event forwarding by
    microseconds.  The SWDGE (gpsimd) ring has a deep software FIFO
    (expansions never stall, events fire at data arrival) but ~2-3.5us
    release-to-first-data latency.  So: HWDGE carries ONLY head+x (sync:
    head, xc; scalar: xa, xb) and the y writes; ALL weights ride SWDGE.
  - The scheduler hoists dependency-free dma_starts to the front of their
    queue, so late emission cannot sequence transfers.  Real WAR gates
    (memset the dest corner; a tiny gpsimd mul reads it, chained off an x
    corner) hold the bulk weight stream back: w1_0 is ungated (m1=1 needs
    it immediately after m1=0 and SWDGE latency is ~3us), w1_1 releases on
    xa, w1_2a.. + w2 + gbg on xb.  Block sizes (1,2,2,2,4,4) m-tiles give
    2-8KB rows: early blocks arrive just-in-time, later fat-row blocks
    sustain 300-400GB/s.
  - m1=0 and m1=1 are interleaved, consuming tiles in arrival order
    (PSUM accumulation order is free): m0 k0-k2 (head/xa), m1 k0-k2
    (w1_0), m0 k5-k7 (xc), m0 k3-k4 (xb), m1 k3-k7 — so the x-wait
    window is filled with real work instead of fillers.
  - y writes ride the (idle during phase B) sync ring; the last d-tile is
    split 384/128 so minimal work remains after the final matmul.

PE warm-up: the HAM clock gate keeps the PE at 1.2 GHz until it has been
busy for a ~3.4us activity window, and any idle gap >3.4us re-throttles it
(idle time freezes ramp progress).  Dummy matmuls on a (gpsimd-memset)
scratch tile start right at "main" and run until the head transfer lands,
so real work begins at 2.4 GHz; filler dummies cover the early data waits.
A dummy activation right after the dma_starts forces both gelu ACT tables
to load at ~6-9us (concurrent, scalar engine idle) instead of mid-phase-A.
"""

import numpy as np

B, N, D, E, F = 8, 512, 1024, 8, 2048
T = B * N
P = 128
CAP = 512            # per-expert token capacity.  Seed-0 max count is 549;
                     # the ~90 overflow tokens (capacity factor 1.0) take
                     # the exact host fallback path.  CAP=512 makes every
                     # matmul a single full-PSUM-bank 512-wide pass.
KT1 = D // P         # 8  k-tiles for matmul1 (contract over D)
MT1 = F // P         # 16 m-tiles for matmul1 / k-tiles for matmul2
DT1 = D // P         # 8  output d-tiles for matmul2
C0 = CAP
CHUNKS = ((0, CAP),)
W1_BLOCKS = (1, 2, 2, 2, 4, 4)       # m1 = 1..15 DMA block sizes
W1_OFF = (1, 2, 4, 6, 8, 12)
W1_XB_GATE = 2       # w1 blocks >= this index release on xb; earlier
                     # blocks release on xa (small early fabric steal,
                     # arrive just-in-time for m1..m3)
X_SPLIT = ((0, 2), (2, 4), (4, 7))   # xa (scalar), xb (scalar), xc (sync)
N_WARM = 16          # upfront dummy matmuls: keep PE busy from ~6.2us
                     # ("main") until head lands (~9.9-11us; sized for
                     # slow-DMA cores) so the HAM clock is at 2.4GHz when
                     # real work starts.
# Gap-filler dummies at phase-A stall points for m1 >= 2 (none needed in
# steady state; the m1=0/1 interleave below has its own hardcoded
# fillers).  {(m1, k): fillers after k-step k, (m1, None): after group}.
FILLERS = {}
W_FILL = 256         # moving width of warm-up/filler dummies

_NC_CACHE = {}


def _build_bass():
    import concourse.bacc as bacc
    import concourse.tile as tile
    from concourse import mybir

    f32 = mybir.dt.float32
    bf16 = mybir.dt.bfloat16

    nc = bacc.Bacc(None, target_bir_lowering=False)
    # head packs xT k-block 0, the whole w1 m1=0 block (all 8 k-tiles), and
    # the b1 bias columns: one early 3.1KB-row transfer covers everything
    # m1=0 needs to start.
    HW = CAP + D + MT1
    head = nc.declare_dram_parameter("head", [P, HW], bf16, isOutput=False)
    xr = nc.declare_dram_parameter("xr", [P, KT1 - 1, CAP], bf16, isOutput=False)
    w1r = nc.declare_dram_parameter("w1r", [P, MT1 - 1, D], bf16, isOutput=False)
    w2r = nc.declare_dram_parameter("w2r", [P, MT1, D], bf16, isOutput=False)
    gbg = nc.declare_dram_parameter("gbg", [P, CAP], f32, isOutput=False)
    y = nc.declare_dram_parameter("y", [D, CAP], bf16, isOutput=True)

    with tile.TileContext(nc) as tc:
        with (
            tc.tile_pool(name="wp", bufs=1) as wp,
            tc.tile_pool(name="hdp", bufs=1) as hdp,
            tc.tile_pool(name="xp", bufs=1) as xp,
            tc.tile_pool(name="w1p", bufs=1) as w1p,
            tc.tile_pool(name="w2p", bufs=1) as w2p,
            tc.tile_pool(name="gp", bufs=1) as gp,
            tc.tile_pool(name="hp", bufs=MT1) as hp,
            tc.tile_pool(name="stp", bufs=3) as stp,
            tc.tile_pool(name="psA", bufs=4, space="PSUM") as psA,
            tc.tile_pool(name="psB", bufs=4, space="PSUM") as psB,
        ):
            # --- PE warm-up: memset + dummy matmuls emitted BEFORE any
            # dma_start, so the memset-done event reaches the PE before the
            # gpsimd sequencer starts expanding SWDGE descriptors. ---
            warm = wp.tile([P, W_FILL], bf16, tag="warm")
            nc.gpsimd.memset(warm[:], 0.0)
            wps = psB.tile([P, C0], f32, tag="psB0", name="warm_ps")
            # tiny psum (borrows a psA slot; phase A's rotation reaches it
            # only at m1=3, long after the dummy act below has read it)
            # feeding the early dummy activation without depending on the
            # warm matmul chain
            tiny = psA.tile([2, 256], f32, tag="psA0", name="tiny_ps")
            nc.tensor.matmul(tiny[:, :], warm[:, 0:2], warm[:],
                             start=True, stop=True, skip_group_check=True)
            for i in range(N_WARM):
                nc.tensor.matmul(wps[:, :W_FILL], warm[:, 0:P], warm[:],
                                 start=True, stop=True, skip_group_check=True)

            # --- DMA plan (see module docstring) ---
            head_t = hdp.tile([P, HW], bf16, tag="head")
            nc.sync.dma_start(out=head_t[:], in_=head[:, :])

            x_t = []
            for j, (a, b) in enumerate(X_SPLIT):
                x_t.append(xp.tile([P, b - a, CAP], bf16, tag=f"x{j}",
                                   name=f"x{j}"))
            nc.scalar.dma_start(out=x_t[0][:], in_=xr[:, 0:2, :])
            nc.scalar.dma_start(out=x_t[1][:], in_=xr[:, 2:4, :])
            nc.sync.dma_start(out=x_t[2][:], in_=xr[:, 4:7, :])

            w1_t = []
            for j, nm in enumerate(W1_BLOCKS):
                w1_t.append(w1p.tile([P, nm, D], bf16, tag=f"w1_{j}",
                                     name=f"w1_{j}"))
            w2_t = w2p.tile([P, MT1, D], bf16, tag="w2")

            def load_w1(j):
                r0 = W1_OFF[j] - 1
                nc.gpsimd.dma_start(out=w1_t[j][:],
                                    in_=w1r[:, r0:r0 + W1_BLOCKS[j], :])

            # Gate the bulk w1/w2/gbg stream on x's arrival so the fabric
            # (~330GB/s under 8-core load) stays mostly head+x's until
            # m1=0's data is in.  A gate must be a REAL dependency: each
            # transfer's destination corner is memset, then a tiny gpsimd
            # mul reads that corner (WAR: the DMA overwrite must wait for
            # the read) chained off the x corner, because the scheduler
            # hoists dependency-free dma_starts to the front of their
            # queue.  SWDGE release-to-first-data latency is ~2-3.5us, so
            # w1_0 (needed by m1=1 right after m1=0) flows UNGATED from the
            # start — its 0.26MB early fabric steal is cheaper than a late
            # m1 entry.  w1_1 releases on xa, everything else on xb.
            gb_sb = gp.tile([P, CAP], f32, tag="gbg")
            load_w1(0)
            gate_seq = [w1_t[j][0:2, 0, 0:2]
                        for j in range(1, len(W1_BLOCKS))]
            gate_seq += [w2_t[0:2, 0, 0:2], gb_sb[0:2, 0:2]]
            for c in gate_seq:
                nc.gpsimd.memset(c, 0.0)
            gate_s = wp.tile([2, 2 * (len(gate_seq) + 1)], f32, tag="gate_s")

            def gate(i, trigger):
                nc.gpsimd.tensor_mul(gate_s[:, 2 * i:2 * i + 2], trigger,
                                     gate_seq[i])

            for j in range(1, len(W1_BLOCKS)):
                i = j - 1
                if j == 1:
                    trig = x_t[0][0:2, 0, 0:2]            # <- xa
                elif j == W1_XB_GATE:
                    trig = x_t[1][0:2, 0, 0:2]            # <- xb
                else:
                    trig = gate_s[:, 2 * i - 2:2 * i]     # chain
                gate(i, trig)
                load_w1(j)
            nw = len(W1_BLOCKS) - 1
            gate(nw, gate_s[:, 2 * nw - 2:2 * nw])
            nc.gpsimd.dma_start(out=w2_t[:], in_=w2r[:, :, :])
            gate(nw + 1, gate_s[:, 2 * nw:2 * nw + 2])
            nc.gpsimd.dma_start(out=gb_sb[:], in_=gbg[:, :])

            # Dummy activation: forces both gelu ACT tables to load on the
            # scalar ENGINE now (~2.6us, concurrent with DMA descriptor
            # expansion on the scalar SEQUENCER), instead of the second
            # table load landing mid-phase-A where it delays event
            # forwarding on the scalar ring.
            gelu = mybir.ActivationFunctionType.Gelu
            act_s = wp.tile([2, 2], f32, tag="act_s")
            nc.scalar.activation(act_s[:, :], tiny[0:2, 0:2], gelu)

            def x_mv(k, a, b):
                if k == 0:
                    return head_t[:, a:b]
                for j, (ka, kb) in enumerate(X_SPLIT):
                    if ka <= k - 1 < kb:
                        return x_t[j][:, k - 1 - ka, a:b]

            def w1_lhs(m1, k):
                if m1 == 0:
                    return head_t[:, CAP + k * P:CAP + (k + 1) * P]
                j = next(i for i in range(len(W1_BLOCKS))
                         if W1_OFF[i] <= m1 < W1_OFF[i] + W1_BLOCKS[i])
                return w1_t[j][:, m1 - W1_OFF[j], k * P:(k + 1) * P]

            # --- Phase A: h^T[F, CAP] = gelu(w1^T @ x^T + b1) ---
            h_sb = []

            def filler(n):
                for _ in range(n):
                    nc.tensor.matmul(wps[:, :W_FILL], warm[:, 0:P], warm[:],
                                     start=True, stop=True,
                                     skip_group_check=True)

            def a_mm(ps, m1, k, st, sp):
                nc.tensor.matmul(ps[:], w1_lhs(m1, k), x_mv(k, 0, CAP),
                                 start=st, stop=sp)

            def a_gelu(ps, m1):
                h = hp.tile([P, CAP], bf16, tag="h", name=f"h_{m1}")
                bias = head_t[:, CAP + D + m1:CAP + D + m1 + 1]
                nc.scalar.activation(h[:, :], ps[:], gelu, bias=bias)
                h_sb.append(h)

            # m1=0 and m1=1 interleaved: while m1=0 waits for the late x
            # pieces (xc ~12.5us, xb ~13us), m1=1's k0..k2 (head/xa data +
            # the ungated w1_0 block, all in SBUF by ~11.3us) run as real
            # work in the wait window instead of fillers.
            ps0 = psA.tile([P, CAP], f32, tag="psA0", name="psA0_0")
            ps1 = psA.tile([P, CAP], f32, tag="psA0", name="psA0_1")
            for k in (0, 1, 2):
                a_mm(ps0, 0, k, k == 0, False)
            filler(4)
            for k in (0, 1, 2):
                a_mm(ps1, 1, k, k == 0, False)
            filler(6)
            for k in (5, 6, 7):
                a_mm(ps0, 0, k, False, False)
            filler(2)
            a_mm(ps0, 0, 3, False, False)
            a_mm(ps0, 0, 4, False, True)
            a_gelu(ps0, 0)
            for k in (3, 4, 5, 6, 7):
                a_mm(ps1, 1, k, False, k == 7)
            a_gelu(ps1, 1)
            filler(2)

            for m1 in range(2, MT1):
                pss = psA.tile([P, CAP], f32, tag="psA0",
                               name=f"psA0_{m1}")
                for i_k, k in enumerate(range(KT1)):
                    a_mm(pss, m1, k, i_k == 0, i_k == KT1 - 1)
                    filler(FILLERS.get((m1, i_k), 0))
                filler(FILLERS.get((m1, None), 0))
                a_gelu(pss, m1)

            # --- Phase B: y^T[D, CAP] = (w2^T @ h^T) * gate ---
            def b_mm(pt, d, k2, a, b):
                lhs = w2_t[:, k2, d * P:(d + 1) * P]
                nc.tensor.matmul(pt[:], lhs, h_sb[k2][:, a:b],
                                 start=(k2 == 0), stop=(k2 == MT1 - 1))

            for d in range(DT1 - 1):
                pss = [psB.tile([P, b - a], f32, tag=f"psB{i}",
                                name=f"psB{i}_b{d}")
                       for i, (a, b) in enumerate(CHUNKS)]
                for k2 in range(MT1):
                    for i, (a, b) in enumerate(CHUNKS):
                        b_mm(pss[i], d, k2, a, b)
                stage = stp.tile([P, CAP], bf16, tag="stage", name=f"st_{d}")
                for i, (a, b) in enumerate(CHUNKS):
                    nc.vector.tensor_mul(stage[:, a:b], pss[i][:],
                                         gb_sb[:, a:b])
                nc.sync.dma_start(out=y[d * P:(d + 1) * P, :], in_=stage[:])
            # last tile: two sequential k2 loops, asymmetric (384/128) so
            # the work remaining after the 384-chunk's write issues is
            # minimal.  Both chunks drain on the sync ring (keeping the
            # scalar sequencer at the end barrier early).
            d = DT1 - 1
            stage = stp.tile([P, CAP], bf16, tag="stage", name=f"st_{d}")
            H = 384
            tail_eng = (nc.sync, nc.sync)
            for i, (a, b) in enumerate(((0, H), (H, CAP))):
                pt = psB.tile([P, CAP], f32, tag="psB0",
                              name=f"psB0_b{d}{i}")
                for k2 in range(MT1):
                    lhs = w2_t[:, k2, d * P:(d + 1) * P]
                    nc.tensor.matmul(pt[:, a:b], lhs, h_sb[k2][:, a:b],
                                     start=(k2 == 0), stop=(k2 == MT1 - 1))
                nc.vector.tensor_mul(stage[:, a:b], pt[:, a:b],
                                     gb_sb[:, a:b])
                tail_eng[i].dma_start(out=y[d * P:(d + 1) * P, a:b],
                                      in_=stage[:, a:b])
    if not nc.is_finalized():
        nc.finalize()
    return nc


def _get_nc():
    if "nc" not in _NC_CACHE:
        _NC_CACHE["nc"] = _build_bass()
    return _NC_CACHE["nc"]


def kernel(x, dispatch_tensor, combine_tensor, w1, b1, w2, b2, **_):
    from concourse.bass_utils import run_bass_kernel_spmd
    from concourse import mybir

    bf = mybir.dt.np(mybir.dt.bfloat16)

    x = np.ascontiguousarray(np.asarray(x, dtype=np.float32)).reshape(T, D)
    dispatch = np.asarray(dispatch_tensor, dtype=np.float32).reshape(T, E)
    combine = np.asarray(combine_tensor, dtype=np.float32).reshape(T, E)
    w1 = np.asarray(w1, dtype=np.float32)
    b1 = np.asarray(b1, dtype=np.float32)
    w2 = np.asarray(w2, dtype=np.float32)
    b2 = np.asarray(b2, dtype=np.float32)

    top = dispatch.argmax(-1)
    gate = combine.sum(-1)
    full = [np.nonzero(top == e)[0] for e in range(E)]
    idxs = [idx[:CAP] for idx in full]
    spill = [idx[CAP:] for idx in full]  # ~90 tokens for the seed-0 routing

    in_maps = []
    for e in range(E):
        idx = idxs[e]
        c = len(idx)
        xT = np.zeros((D, CAP), bf)
        xT[:, :c] = x[idx].T.astype(bf)
        # w1s[m1, p, k*P+m] = w1[k*P+p, m1*P+m]: per-m1 [P, D] blocks whose
        # [:, k*P:(k+1)*P] slice is the lhsT k-tile for output tile m1.
        w1s = np.ascontiguousarray(
            w1[e].reshape(KT1, P, MT1, P).transpose(2, 1, 0, 3)
        ).astype(bf).reshape(MT1, P, D)
        # w2s[k2, p, d] = w2[k2*P+p, d]: lhsT tiles for phase B.
        w2s = np.ascontiguousarray(
            w2[e].reshape(MT1, P, D)).astype(bf).transpose(1, 0, 2)
        g = np.zeros(CAP, np.float32)
        g[:c] = gate[idx]
        gbgv = np.ascontiguousarray(
            np.broadcast_to(g[None, :], (P, CAP)))
        b1t = b1[e].reshape(MT1, P).T.astype(bf)  # [P, MT1] bias columns
        in_maps.append({
            "head": np.ascontiguousarray(
                np.concatenate([xT[:P], w1s[0], b1t], axis=1)),
            "xr": np.ascontiguousarray(
                xT[P:].reshape(KT1 - 1, P, CAP).transpose(1, 0, 2)),
            "w1r": np.ascontiguousarray(w1s[1:].transpose(1, 0, 2)),
            "w2r": np.ascontiguousarray(w2s),
            "gbg": gbgv,
        })

    global _LAST_IN_MAPS
    _LAST_IN_MAPS = in_maps
    nc = _get_nc()
    res = run_bass_kernel_spmd(nc, in_maps, list(range(E)))
    ys = [np.asarray(res.results[e]["y"], dtype=np.float32) for e in range(E)]
    if not all(np.isfinite(ye).all() for ye in ys):
        # rare transient device glitch observed once in ~20 runs: retry once
        res = run_bass_kernel_spmd(nc, in_maps, list(range(E)))
        ys = [np.asarray(res.results[e]["y"], dtype=np.float32)
              for e in range(E)]

    y_flat = np.empty((T, D), np.float32)
    for e in range(E):
        c = len(idxs[e])
        y_flat[idxs[e]] = ys[e].T[:c]
        if len(spill[e]):
            # capacity-overflow tokens: exact fp32 math on host
            import math

            erf = np.frompyfunc(math.erf, 1, 1)
            hs = x[spill[e]] @ w1[e] + b1[e]
            hs = hs * 0.5 * (1.0 + erf(hs / np.sqrt(2.0)).astype(np.float64))
            y_flat[spill[e]] = ((hs.astype(np.float32) @ w2[e])
                                * gate[spill[e]][:, None])
    return (y_flat + b2[None, :]).reshape(B, N, D)
